# revision 18
# baseline (speedup 1.0000x reference)
"""Trainium2 kernel for nn_KerasDense_32263794328408.

y = relu(x @ M + b), where M is a 4096x4096 TT-matrix (cores of shape
[r_{k-1}, 8, 8, r_k], ranks [1,8,8,8,1]).

Strategy: the TT cores are tiny (<17 KB each); materialize the dense
M = 4096x4096 on the host (cheap, ~270 MFLOP) and run the dense
y = relu(x @ M + b) as a near-roofline GEMM on 8 NeuronCores.

Sharding: 2D grid, 4 batch groups x 2 output-column groups.
Per core: x-shard [1024, 4096] (shipped transposed as xT [4096, 1024]),
W column-half [4096, 2048] and bias half, producing y [1024, 2048].

Inputs are shipped in bf16 (quantization absmax/scale ~2.5e-3, well
under the 2e-2 gate). vs fp32r this halves HBM traffic — the f32
version was pinned at the per-core HBM ceiling during the x-load phase
and stalled the PE — and bf16 LDWEIGHTS gets fast-weight-load, so the
stationary reload hides fully under the matmul stream.

On-chip: x-stationary matmuls. lhsT = xT tile [128k x 128b], rhs = W
slab [128k x 512o], PSUM accumulation over the 32 k-tiles plus a K=1
matmul adding the bias. W is host-pre-blocked so every slab DMA is a
fully contiguous [128, 2048] read (4 KB per partition line). The last
k-quad of each output chunk runs bt-major with the bias matmul and the
DVE relu+store chained per batch tile, so PSUM banks free one by one
and the drains overlap the next chunk's matmuls (short kernel tail).
"""

import sys

if "/opt/trn_rl_repo" not in sys.path:
    sys.path.insert(0, "/opt/trn_rl_repo")

import ml_dtypes
import numpy as np

import concourse.bacc as bacc
import concourse.bass as bass
import concourse.mybir as mybir
import concourse.tile as tile
from concourse.bass_utils import run_bass_kernel_spmd

F32 = mybir.dt.float32
BF16 = mybir.dt.bfloat16
NP_BF16 = ml_dtypes.bfloat16

B_FULL = 4096  # batch
F_FULL = 4096  # input features
O_FULL = 4096  # output features

BG = 4  # batch groups
OG = 2  # output-column groups
N_CORES = BG * OG

B_L = B_FULL // BG   # 1024 batch rows per core
O_L = O_FULL // OG   # 2048 output cols per core
KT = F_FULL // 128   # 32 contraction tiles
OC = O_L // 512      # 4 output chunks of 512 per core
BT = B_L // 128      # 8 batch tiles of 128 per core
KQ = 4               # k-tiles fetched per W DMA
NQ = KT // KQ        # 8 quad fetches per oc

_CACHE: dict = {}


def _build_module() -> bass.Bass:
    nc = bacc.Bacc(None, target_bir_lowering=False)

    xT = nc.declare_dram_parameter("xT", [F_FULL, B_L], BF16, isOutput=False)
    # w is host-pre-blocked: row si*128+p, col k4*512+c holds
    # W[(ktq*4+k4)*128 + p, oc*512 + c] with si = oc*NQ + ktq, so each
    # slab (oc, ktq) is one contiguous [128, 2048] read.
    w = nc.declare_dram_parameter("w", [F_FULL, O_L], BF16, isOutput=False)
    bvec = nc.declare_dram_parameter("bvec", [1, O_L], BF16, isOutput=False)
    ones = nc.declare_dram_parameter("ones", [1, 128], BF16, isOutput=False)
    bbc = nc.declare_dram_parameter("bbc", [128, O_L], F32, isOutput=False)
    y = nc.declare_dram_parameter("y", [B_L, O_L], F32, isOutput=True)

    with tile.TileContext(nc) as tc:
        with (
            tc.tile_pool(name="xt", bufs=1) as xt_pool,
            tc.tile_pool(name="w0", bufs=1) as w0_pool,
            tc.tile_pool(name="wsl", bufs=4) as w_pool,
            tc.tile_pool(name="yst", bufs=12) as y_pool,
            tc.tile_pool(name="cst", bufs=1) as c_pool,
            tc.tile_pool(name="acc", bufs=8, space="PSUM") as psum_pool,
        ):
            # Tiny constants first so they land with queue startup.
            ones_sb = c_pool.tile([1, 128], BF16, tag="ones")
            nc.scalar.dma_start(out=ones_sb[:], in_=ones[:])
            bias_sb = c_pool.tile([1, O_L], BF16, tag="bias")
            nc.scalar.dma_start(out=bias_sb[:], in_=bvec[:])

            # The first two W quads as separate [128, 512] tiles so early
            # matmuls only wait on 128 KB loads (the cold-start window is
            # paced by these arrivals).
            w0q = {}
            for ktq in range(2):
                for k4 in range(KQ):
                    t = w0_pool.tile([128, 512], BF16, tag=f"w0_{ktq}_{k4}",
                                     name=f"w0_{ktq}_{k4}")
                    nc.sync.dma_start(
                        out=t[:],
                        in_=w[ktq * 128 : (ktq + 1) * 128,
                              k4 * 512 : (k4 + 1) * 512],
                    )
                    w0q[(ktq, k4)] = t

            # xT resident in SBUF, one tile per k-tile so consumers only
            # wait on their own 256 KB load. ACT HWDGE ring.
            xts = []
            for kt in range(KT):
                t = xt_pool.tile([128, B_L], BF16, tag=f"xt{kt}", name=f"xt{kt}")
                nc.scalar.dma_start(out=t[:], in_=xT[kt * 128 : (kt + 1) * 128, :])
                xts.append(t)

            # Broadcast bias rows for the DVE drain; first needed when the
            # first output chunk finishes (~60 us), well after the x tiles
            # ahead of it on this ring.
            bbc_sb = c_pool.tile([128, O_L], F32, tag="bbc")
            nc.scalar.dma_start(out=bbc_sb[:], in_=bbc[:])

            # Warm-up matmuls on the tiny bias constant: the PE HAM clock
            # gate needs ~3.4 us of sustained activity to lift the cold
            # 1.2 GHz throttle, and the first real matmul can't start until
            # its x/W tiles land (~12 us: NEFF preamble + first loads).
            # Burning that idle window on dummy K=1 matmuls makes the real
            # stream run warm from its first instruction.
            warm = psum_pool.tile([128, 512], F32, tag="acc", name="warm")
            for i in range(8):
                nc.tensor.matmul(
                    warm[:],
                    ones_sb[:],
                    bias_sb[:, 0:512],
                    start=(i == 0),
                    stop=(i == 7),
                )

            for oc in range(OC):
                accs = [
                    psum_pool.tile([128, 512], F32, tag="acc",
                                   name=f"acc_{oc}_{bt}")
                    for bt in range(BT)
                ]
                for ktq in range(NQ):
                    si = oc * NQ + ktq
                    if oc == 0 and ktq < 2:
                        wchunks = [w0q[(ktq, k4)][:] for k4 in range(KQ)]
                    else:
                        w_sl = w_pool.tile([128, KQ * 512], BF16, tag="wsl",
                                           name=f"w_{oc}_{ktq}")
                        nc.sync.dma_start(
                            out=w_sl[:], in_=w[si * 128 : (si + 1) * 128, :]
                        )
                        wchunks = [
                            w_sl[:, k4 * 512 : (k4 + 1) * 512] for k4 in range(KQ)
                        ]
                    if ktq < NQ - 1:
                        for k4 in range(KQ):
                            kt = ktq * KQ + k4
                            for bt in range(BT):
                                nc.tensor.matmul(
                                    accs[bt][:],
                                    xts[kt][:, bt * 128 : (bt + 1) * 128],
                                    wchunks[k4],
                                    start=(kt == 0),
                                    stop=False,
                                )
                    else:
                        # Last quad bt-major: each batch tile finishes its
                        # contraction and drains while the PE moves on —
                        # banks free one at a time, drains and stores
                        # overlap the next oc's matmuls.
                        #
                        # Mid-kernel, bias + relu happen on the (otherwise
                        # idle) DVE instead of costing PE matmuls, and the
                        # y stores stay OFF the sync queue: HWDGE queues
                        # are strictly in-order, so a store parked behind
                        # an unmet semaphore would block the next chunk's
                        # W-slab fetch behind it and stall the PE.
                        #
                        # For the final chunk there are no W fetches left
                        # to protect, but the drain chain IS the kernel
                        # tail — so spend 8 K=1 matmuls on the bias to keep
                        # the DVE drain single-op (0.7us < 0.86us matmul
                        # spacing) and split the stores across both queues.
                        last_oc = oc == OC - 1
                        y_sls = []
                        for bt in range(BT):
                            for k4 in range(KQ):
                                kt = ktq * KQ + k4
                                nc.tensor.matmul(
                                    accs[bt][:],
                                    xts[kt][:, bt * 128 : (bt + 1) * 128],
                                    wchunks[k4],
                                    start=False,
                                    stop=(not last_oc and kt == KT - 1),
                                )
                            y_sl = y_pool.tile([128, 512], F32, tag="yst",
                                               name=f"y_{oc}_{bt}")
                            y_sls.append(y_sl)
                            if last_oc:
                                nc.tensor.matmul(
                                    accs[bt][:],
                                    ones_sb[:],
                                    bias_sb[:, oc * 512 : (oc + 1) * 512],
                                    start=False,
                                    stop=True,
                                )
                                if bt < BT - 1:
                                    nc.vector.tensor_scalar_max(
                                        y_sl[:], accs[bt][:], 0.0
                                    )
                                    dma_eng = (
                                        nc.scalar if bt % 2 == 0 else nc.sync
                                    )
                                    dma_eng.dma_start(
                                        out=y[
                                            bt * 128 : (bt + 1) * 128,
                                            oc * 512 : (oc + 1) * 512,
                                        ],
                                        in_=y_sl[:],
                                    )
                                else:
                                    # Very last tile: halve the relu+store
                                    # and fan across both queues — this
                                    # chain IS the kernel tail.
                                    for h, dma_eng in enumerate(
                                        (nc.sync, nc.scalar)
                                    ):
                                        cols = slice(h * 256, (h + 1) * 256)
                                        nc.vector.tensor_scalar_max(
                                            y_sl[:, cols], accs[bt][:, cols], 0.0
                                        )
                                        dma_eng.dma_start(
                                            out=y[
                                                bt * 128 : (bt + 1) * 128,
                                                oc * 512 + h * 256 : oc * 512
                                                + (h + 1) * 256,
                                            ],
                                            in_=y_sl[:, cols],
                                        )
                            else:
                                # Bias-add on DVE frees the PSUM bank in
                                # 0.7us — under the 0.86us/bt matmul pace,
                                # so the next chunk's matmuls never wait on
                                # a bank. Only the adds sit on the DVE
                                # during the handoff; relu + store follow
                                # below once all banks are clear.
                                nc.vector.tensor_add(
                                    y_sl[:],
                                    accs[bt][:],
                                    bbc_sb[:, oc * 512 : (oc + 1) * 512],
                                )
                        if not last_oc:
                            for bt in range(BT):
                                nc.vector.tensor_scalar_max(
                                    y_sls[bt][:], y_sls[bt][:], 0.0
                                )
                                nc.scalar.dma_start(
                                    out=y[
                                        bt * 128 : (bt + 1) * 128,
                                        oc * 512 : (oc + 1) * 512,
                                    ],
                                    in_=y_sls[bt][:],
                                )

    nc.finalize()
    return nc


def _materialize_w(core0, core1, core2, core3) -> np.ndarray:
    """Contract the TT cores into the dense 4096x4096 matrix M.

    M[(m1 m2 m3 m4), (n1 n2 n3 n4)] (big-endian mode order on both sides),
    matching the reference's x/y index conventions.
    """
    g1 = core0[0].astype(np.float64)            # [m1, n1, r1]
    t12 = np.einsum("mnr,rMNs->mMnNs", g1, core1.astype(np.float64))
    a12 = t12.reshape(64, 64, 8)                # [(m1 m2), (n1 n2), r2]
    g4 = core3[..., 0].astype(np.float64)       # [r3, m4, n4]
    t34 = np.einsum("rmns,sMN->rmMnN", core2.astype(np.float64), g4)
    b34 = t34.reshape(8, 64, 64)                # [r2, (m3 m4), (n3 n4)]
    w = np.einsum("mnr,rMN->mMnN", a12, b34)    # [(m12),(m34),(n12),(n34)]
    return np.ascontiguousarray(
        w.reshape(F_FULL, O_FULL), dtype=np.float32
    )


def _prepare_in_maps(x, w_full, bias):
    """Shard + bf16-quantize host-side. Core c = (g, h): batch group g,
    output-column group h."""
    xts = [
        np.ascontiguousarray(
            x[g * B_L : (g + 1) * B_L, :].T.astype(NP_BF16)
        )
        for g in range(BG)
    ]
    ws = []
    for h in range(OG):
        wh = w_full[:, h * O_L : (h + 1) * O_L].astype(NP_BF16)
        t = wh.reshape(NQ, KQ, 128, OC, 512)      # [ktq, k4, p, oc, c]
        wb = np.ascontiguousarray(t.transpose(3, 0, 2, 1, 4)).reshape(
            F_FULL, O_L
        )                                          # [(oc ktq p), (k4 c)]
        ws.append(wb)
    bs = [
        np.ascontiguousarray(bias[:, h * O_L : (h + 1) * O_L].astype(NP_BF16))
        for h in range(OG)
    ]
    bbcs = [
        np.ascontiguousarray(
            np.broadcast_to(bias[:, h * O_L : (h + 1) * O_L], (128, O_L))
        ).astype(np.float32)
        for h in range(OG)
    ]
    ones = np.ones((1, 128), dtype=NP_BF16)
    in_maps = []
    for c in range(N_CORES):
        g, h = divmod(c, OG)
        in_maps.append(
            {"xT": xts[g], "w": ws[h], "bvec": bs[h], "ones": ones, "bbc": bbcs[h]}
        )
    return in_maps


def kernel(x, core0, core1, core2, core3, b) -> np.ndarray:
    x = np.asarray(x, dtype=np.float32)
    w_full = _materialize_w(
        np.asarray(core0, dtype=np.float32),
        np.asarray(core1, dtype=np.float32),
        np.asarray(core2, dtype=np.float32),
        np.asarray(core3, dtype=np.float32),
    )
    bias = np.asarray(b, dtype=np.float32).reshape(1, O_FULL)

    if "nc" not in _CACHE:
        _CACHE["nc"] = _build_module()
    nc = _CACHE["nc"]

    in_maps = _prepare_in_maps(x, w_full, bias)
    res = run_bass_kernel_spmd(nc, in_maps, core_ids=list(range(N_CORES)))

    y = np.empty((B_FULL, O_FULL), dtype=np.float32)
    for c in range(N_CORES):
        g, h = divmod(c, OG)
        y[g * B_L : (g + 1) * B_L, h * O_L : (h + 1) * O_L] = res.results[c]["y"]
    return y


# revision 23
# speedup vs baseline: 1.1970x; 1.1970x over previous
"""Trainium2 kernel for nn_KerasDense_32263794328408.

y = relu(x @ M + b), where M is a 4096x4096 TT-matrix (cores of shape
[r_{k-1}, 8, 8, r_k], ranks [1,8,8,8,1]).

Strategy: the TT cores are tiny (<17 KB each); materialize the dense
M = 4096x4096 on the host (cheap, ~270 MFLOP) and run the dense
y = relu(x @ M + b) as a near-roofline GEMM on 8 NeuronCores.

Sharding: 2D grid, 4 batch groups x 2 output-column groups.
Per core: x-shard [1024, 4096] (shipped transposed as xT [4096, 1024]),
W column-half [4096, 2048] and bias half, producing y [1024, 2048].

Inputs are shipped in bf16 (quantization absmax/scale ~2.5e-3, well
under the 2e-2 gate). vs fp32r this halves HBM traffic — the f32
version was pinned at the per-core HBM ceiling during the x-load phase
and stalled the PE — and bf16 LDWEIGHTS gets fast-weight-load, so the
stationary reload hides fully under the matmul stream.

On-chip: x-stationary matmuls. lhsT = xT tile [128k x 128b], rhs = W
slab [128k x 512o], PSUM accumulation over the 32 k-tiles plus a K=1
matmul adding the bias. W is host-pre-blocked so every slab DMA is a
fully contiguous [128, 2048] read (4 KB per partition line). The last
k-quad of each output chunk runs bt-major with the bias matmul and the
DVE relu+store chained per batch tile, so PSUM banks free one by one
and the drains overlap the next chunk's matmuls (short kernel tail).
"""

import sys

if "/opt/trn_rl_repo" not in sys.path:
    sys.path.insert(0, "/opt/trn_rl_repo")

import ml_dtypes
import numpy as np

import concourse.bacc as bacc
import concourse.bass as bass
import concourse.mybir as mybir
import concourse.tile as tile
from concourse.bass_utils import run_bass_kernel_spmd

F32 = mybir.dt.float32
BF16 = mybir.dt.bfloat16
NP_BF16 = ml_dtypes.bfloat16

B_FULL = 4096  # batch
F_FULL = 4096  # input features
O_FULL = 4096  # output features

BG = 4  # batch groups
OG = 2  # output-column groups
N_CORES = BG * OG

B_L = B_FULL // BG   # 1024 batch rows per core
O_L = O_FULL // OG   # 2048 output cols per core
KT = F_FULL // 128   # 32 contraction tiles
OC = O_L // 512      # 4 output chunks of 512 per core
BT = B_L // 128      # 8 batch tiles of 128 per core
KQ = 4               # k-tiles fetched per W DMA
NQ = KT // KQ        # 8 quad fetches per oc

_CACHE: dict = {}


def _build_module() -> bass.Bass:
    nc = bacc.Bacc(None, target_bir_lowering=False)

    xT = nc.declare_dram_parameter("xT", [F_FULL, B_L], BF16, isOutput=False)
    # w is host-pre-blocked: row si*128+p, col k4*512+c holds
    # W[(ktq*4+k4)*128 + p, oc*512 + c] with si = oc*NQ + ktq, so each
    # slab (oc, ktq) is one contiguous [128, 2048] read.
    w = nc.declare_dram_parameter("w", [F_FULL, O_L], BF16, isOutput=False)
    bvec = nc.declare_dram_parameter("bvec", [1, O_L], BF16, isOutput=False)
    ones = nc.declare_dram_parameter("ones", [128, 640], BF16, isOutput=False)
    bbc = nc.declare_dram_parameter("bbc", [128, O_L], F32, isOutput=False)
    y = nc.declare_dram_parameter("y", [B_L, O_L], F32, isOutput=True)

    with tile.TileContext(nc) as tc:
        with (
            tc.tile_pool(name="xt", bufs=1) as xt_pool,
            tc.tile_pool(name="w0", bufs=1) as w0_pool,
            tc.tile_pool(name="wsl", bufs=4) as w_pool,
            tc.tile_pool(name="yst", bufs=12) as y_pool,
            tc.tile_pool(name="cst", bufs=1) as c_pool,
            tc.tile_pool(name="acc", bufs=8, space="PSUM") as psum_pool,
        ):
            # Tiny constants first so they land with queue startup.
            ones_sb = c_pool.tile([128, 640], BF16, tag="ones")
            nc.scalar.dma_start(out=ones_sb[:], in_=ones[:])
            bias_sb = c_pool.tile([1, O_L], BF16, tag="bias")
            nc.scalar.dma_start(out=bias_sb[:], in_=bvec[:])

            # The first two W quads as separate [128, 512] tiles so early
            # matmuls only wait on 128 KB loads (the cold-start window is
            # paced by these arrivals).
            w0q = {}
            for ktq in range(2):
                for k4 in range(KQ):
                    t = w0_pool.tile([128, 512], BF16, tag=f"w0_{ktq}_{k4}",
                                     name=f"w0_{ktq}_{k4}")
                    nc.sync.dma_start(
                        out=t[:],
                        in_=w[ktq * 128 : (ktq + 1) * 128,
                              k4 * 512 : (k4 + 1) * 512],
                    )
                    w0q[(ktq, k4)] = t

            # xT resident in SBUF, one tile per k-tile so consumers only
            # wait on their own 256 KB load. ACT HWDGE ring.
            xts = []
            for kt in range(KT):
                t = xt_pool.tile([128, B_L], BF16, tag=f"xt{kt}", name=f"xt{kt}")
                nc.scalar.dma_start(out=t[:], in_=xT[kt * 128 : (kt + 1) * 128, :])
                xts.append(t)

            # Broadcast bias rows for the DVE drain; first needed when the
            # first output chunk finishes (~60 us), well after the x tiles
            # ahead of it on this ring.
            bbc_sb = c_pool.tile([128, O_L], F32, tag="bbc")
            nc.scalar.dma_start(out=bbc_sb[:], in_=bbc[:])

            # Warm-up matmuls on the tiny bias constant: the PE HAM clock
            # gate needs ~3.4 us of sustained activity to lift the cold
            # 1.2 GHz throttle, and the first real matmul can't start until
            # its x/W tiles land (~12 us: NEFF preamble + first loads).
            # Burning that idle window on dummy K=1 matmuls makes the real
            # stream run warm from its first instruction.
            # Full-K warmups: K=1 matmuls light only 1 of 128 PE rows and
            # barely register with the HAM activity monitor (flip came ~10us
            # after they started); full 128-row matmuls flip it on schedule.
            warm = psum_pool.tile([128, 512], F32, tag="acc", name="warm")
            for i in range(8):
                nc.tensor.matmul(
                    warm[:],
                    ones_sb[:, 0:128],
                    ones_sb[:, 128:640],
                    start=(i == 0),
                    stop=(i == 7),
                )

            for oc in range(OC):
                accs = [
                    psum_pool.tile([128, 512], F32, tag="acc",
                                   name=f"acc_{oc}_{bt}")
                    for bt in range(BT)
                ]
                for ktq in range(NQ):
                    si = oc * NQ + ktq
                    if oc == 0 and ktq < 2:
                        wchunks = [w0q[(ktq, k4)][:] for k4 in range(KQ)]
                    else:
                        w_sl = w_pool.tile([128, KQ * 512], BF16, tag="wsl",
                                           name=f"w_{oc}_{ktq}")
                        nc.sync.dma_start(
                            out=w_sl[:], in_=w[si * 128 : (si + 1) * 128, :]
                        )
                        wchunks = [
                            w_sl[:, k4 * 512 : (k4 + 1) * 512] for k4 in range(KQ)
                        ]
                    if ktq < NQ - 1:
                        for k4 in range(KQ):
                            kt = ktq * KQ + k4
                            for bt in range(BT):
                                nc.tensor.matmul(
                                    accs[bt][:],
                                    xts[kt][:, bt * 128 : (bt + 1) * 128],
                                    wchunks[k4],
                                    start=(kt == 0),
                                    stop=False,
                                )
                    else:
                        # Last quad bt-major: each batch tile finishes its
                        # contraction and drains while the PE moves on —
                        # banks free one at a time, drains and stores
                        # overlap the next oc's matmuls.
                        #
                        # Mid-kernel, bias + relu happen on the (otherwise
                        # idle) DVE instead of costing PE matmuls, and the
                        # y stores stay OFF the sync queue: HWDGE queues
                        # are strictly in-order, so a store parked behind
                        # an unmet semaphore would block the next chunk's
                        # W-slab fetch behind it and stall the PE.
                        #
                        # For the final chunk there are no W fetches left
                        # to protect, but the drain chain IS the kernel
                        # tail — so spend 8 K=1 matmuls on the bias to keep
                        # the DVE drain single-op (0.7us < 0.86us matmul
                        # spacing) and split the stores across both queues.
                        last_oc = oc == OC - 1
                        y_sls = []
                        for bt in range(BT):
                            for k4 in range(KQ):
                                kt = ktq * KQ + k4
                                nc.tensor.matmul(
                                    accs[bt][:],
                                    xts[kt][:, bt * 128 : (bt + 1) * 128],
                                    wchunks[k4],
                                    start=False,
                                    stop=(not last_oc and kt == KT - 1),
                                )
                            y_sl = y_pool.tile([128, 512], F32, tag="yst",
                                               name=f"y_{oc}_{bt}")
                            y_sls.append(y_sl)
                            if last_oc:
                                nc.tensor.matmul(
                                    accs[bt][:],
                                    ones_sb[0:1, 0:128],
                                    bias_sb[:, oc * 512 : (oc + 1) * 512],
                                    start=False,
                                    stop=True,
                                )
                                if bt < BT - 1:
                                    nc.vector.tensor_scalar_max(
                                        y_sl[:], accs[bt][:], 0.0
                                    )
                                    dma_eng = (
                                        nc.scalar if bt % 2 == 0 else nc.sync
                                    )
                                    dma_eng.dma_start(
                                        out=y[
                                            bt * 128 : (bt + 1) * 128,
                                            oc * 512 : (oc + 1) * 512,
                                        ],
                                        in_=y_sl[:],
                                    )
                                else:
                                    # Very last tile: halve the relu+store
                                    # and fan across both queues — this
                                    # chain IS the kernel tail.
                                    for h, dma_eng in enumerate(
                                        (nc.sync, nc.scalar)
                                    ):
                                        cols = slice(h * 256, (h + 1) * 256)
                                        nc.vector.tensor_scalar_max(
                                            y_sl[:, cols], accs[bt][:, cols], 0.0
                                        )
                                        dma_eng.dma_start(
                                            out=y[
                                                bt * 128 : (bt + 1) * 128,
                                                oc * 512 + h * 256 : oc * 512
                                                + (h + 1) * 256,
                                            ],
                                            in_=y_sl[:, cols],
                                        )
                            else:
                                # Bias-add on DVE frees the PSUM bank in
                                # 0.7us — under the 0.86us/bt matmul pace,
                                # so the next chunk's matmuls never wait on
                                # a bank. Only the adds sit on the DVE
                                # during the handoff; relu + store follow
                                # below once all banks are clear.
                                nc.vector.tensor_add(
                                    y_sl[:],
                                    accs[bt][:],
                                    bbc_sb[:, oc * 512 : (oc + 1) * 512],
                                )
                        if not last_oc:
                            for bt in range(BT):
                                nc.vector.tensor_scalar_max(
                                    y_sls[bt][:], y_sls[bt][:], 0.0
                                )
                                nc.scalar.dma_start(
                                    out=y[
                                        bt * 128 : (bt + 1) * 128,
                                        oc * 512 : (oc + 1) * 512,
                                    ],
                                    in_=y_sls[bt][:],
                                )

    nc.finalize()
    return nc


def _materialize_w(core0, core1, core2, core3) -> np.ndarray:
    """Contract the TT cores into the dense 4096x4096 matrix M.

    M[(m1 m2 m3 m4), (n1 n2 n3 n4)] (big-endian mode order on both sides),
    matching the reference's x/y index conventions.
    """
    g1 = core0[0].astype(np.float64)            # [m1, n1, r1]
    t12 = np.einsum("mnr,rMNs->mMnNs", g1, core1.astype(np.float64))
    a12 = t12.reshape(64, 64, 8)                # [(m1 m2), (n1 n2), r2]
    g4 = core3[..., 0].astype(np.float64)       # [r3, m4, n4]
    t34 = np.einsum("rmns,sMN->rmMnN", core2.astype(np.float64), g4)
    b34 = t34.reshape(8, 64, 64)                # [r2, (m3 m4), (n3 n4)]
    w = np.einsum("mnr,rMN->mMnN", a12, b34)    # [(m12),(m34),(n12),(n34)]
    return np.ascontiguousarray(
        w.reshape(F_FULL, O_FULL), dtype=np.float32
    )


def _prepare_in_maps(x, w_full, bias):
    """Shard + bf16-quantize host-side. Core c = (g, h): batch group g,
    output-column group h."""
    xts = [
        np.ascontiguousarray(
            x[g * B_L : (g + 1) * B_L, :].T.astype(NP_BF16)
        )
        for g in range(BG)
    ]
    ws = []
    for h in range(OG):
        wh = w_full[:, h * O_L : (h + 1) * O_L].astype(NP_BF16)
        t = wh.reshape(NQ, KQ, 128, OC, 512)      # [ktq, k4, p, oc, c]
        wb = np.ascontiguousarray(t.transpose(3, 0, 2, 1, 4)).reshape(
            F_FULL, O_L
        )                                          # [(oc ktq p), (k4 c)]
        ws.append(wb)
    bs = [
        np.ascontiguousarray(bias[:, h * O_L : (h + 1) * O_L].astype(NP_BF16))
        for h in range(OG)
    ]
    bbcs = [
        np.ascontiguousarray(
            np.broadcast_to(bias[:, h * O_L : (h + 1) * O_L], (128, O_L))
        ).astype(np.float32)
        for h in range(OG)
    ]
    ones = np.ones((128, 640), dtype=NP_BF16)
    in_maps = []
    for c in range(N_CORES):
        g, h = divmod(c, OG)
        in_maps.append(
            {"xT": xts[g], "w": ws[h], "bvec": bs[h], "ones": ones, "bbc": bbcs[h]}
        )
    return in_maps


def kernel(x, core0, core1, core2, core3, b) -> np.ndarray:
    x = np.asarray(x, dtype=np.float32)
    w_full = _materialize_w(
        np.asarray(core0, dtype=np.float32),
        np.asarray(core1, dtype=np.float32),
        np.asarray(core2, dtype=np.float32),
        np.asarray(core3, dtype=np.float32),
    )
    bias = np.asarray(b, dtype=np.float32).reshape(1, O_FULL)

    if "nc" not in _CACHE:
        _CACHE["nc"] = _build_module()
    nc = _CACHE["nc"]

    in_maps = _prepare_in_maps(x, w_full, bias)
    res = run_bass_kernel_spmd(nc, in_maps, core_ids=list(range(N_CORES)))

    y = np.empty((B_FULL, O_FULL), dtype=np.float32)
    for c in range(N_CORES):
        g, h = divmod(c, OG)
        y[g * B_L : (g + 1) * B_L, h * O_L : (h + 1) * O_L] = res.results[c]["y"]
    return y


# revision 29
# speedup vs baseline: 1.2054x; 1.0069x over previous
"""Trainium2 kernel for nn_KerasDense_32263794328408.

y = relu(x @ M + b), where M is a 4096x4096 TT-matrix (cores of shape
[r_{k-1}, 8, 8, r_k], ranks [1,8,8,8,1]).

Strategy: the TT cores are tiny (<17 KB each); materialize the dense
M = 4096x4096 on the host (cheap, ~270 MFLOP) and run the dense
y = relu(x @ M + b) as a near-roofline GEMM on 8 NeuronCores.

Sharding: 2D grid, 4 batch groups x 2 output-column groups.
Per core: x-shard [1024, 4096] (shipped transposed as xT [4096, 1024]),
W column-half [4096, 2048] and bias half, producing y [1024, 2048].

Inputs are shipped in bf16 (quantization absmax/scale ~2.5e-3, well
under the 2e-2 gate). vs fp32r this halves HBM traffic — the f32
version was pinned at the per-core HBM ceiling during the x-load phase
and stalled the PE — and bf16 LDWEIGHTS gets fast-weight-load, so the
stationary reload hides fully under the matmul stream.

On-chip: x-stationary matmuls. lhsT = xT tile [128k x 128b], rhs = W
slab [128k x 512o], PSUM accumulation over the 32 k-tiles plus a K=1
matmul adding the bias. W is host-pre-blocked so every slab DMA is a
fully contiguous [128, 2048] read (4 KB per partition line). The last
k-quad of each output chunk runs bt-major with the bias matmul and the
DVE relu+store chained per batch tile, so PSUM banks free one by one
and the drains overlap the next chunk's matmuls (short kernel tail).
"""

import sys

if "/opt/trn_rl_repo" not in sys.path:
    sys.path.insert(0, "/opt/trn_rl_repo")

import ml_dtypes
import numpy as np

import concourse.bacc as bacc
import concourse.bass as bass
import concourse.mybir as mybir
import concourse.tile as tile
from concourse.bass_utils import run_bass_kernel_spmd

F32 = mybir.dt.float32
BF16 = mybir.dt.bfloat16
NP_BF16 = ml_dtypes.bfloat16

B_FULL = 4096  # batch
F_FULL = 4096  # input features
O_FULL = 4096  # output features

BG = 4  # batch groups
OG = 2  # output-column groups
N_CORES = BG * OG

B_L = B_FULL // BG   # 1024 batch rows per core
O_L = O_FULL // OG   # 2048 output cols per core
KT = F_FULL // 128   # 32 contraction tiles
OC = O_L // 512      # 4 output chunks of 512 per core
BT = B_L // 128      # 8 batch tiles of 128 per core
KQ = 4               # k-tiles fetched per W DMA
NQ = KT // KQ        # 8 quad fetches per oc

_CACHE: dict = {}


def _build_module() -> bass.Bass:
    nc = bacc.Bacc(None, target_bir_lowering=False)

    xT = nc.declare_dram_parameter("xT", [F_FULL, B_L], BF16, isOutput=False)
    # w is host-pre-blocked: row si*128+p, col k4*512+c holds
    # W[(ktq*4+k4)*128 + p, oc*512 + c] with si = oc*NQ + ktq, so each
    # slab (oc, ktq) is one contiguous [128, 2048] read.
    w = nc.declare_dram_parameter("w", [F_FULL, O_L], BF16, isOutput=False)
    bvec = nc.declare_dram_parameter("bvec", [1, O_L], BF16, isOutput=False)
    ones = nc.declare_dram_parameter("ones", [128, 128], BF16, isOutput=False)
    bbc = nc.declare_dram_parameter("bbc", [128, O_L], F32, isOutput=False)
    # y ships back as bf16 (host upcasts): halves store traffic and the
    # tail store; costs ~2e-3 extra absmax, still 5x under the gate.
    y = nc.declare_dram_parameter("y", [B_L, O_L], BF16, isOutput=True)

    with tile.TileContext(nc) as tc:
        with (
            tc.tile_pool(name="xt", bufs=1) as xt_pool,
            tc.tile_pool(name="w0", bufs=1) as w0_pool,
            tc.tile_pool(name="wsl", bufs=6) as w_pool,
            tc.tile_pool(name="yst", bufs=12) as y_pool,
            tc.tile_pool(name="cst", bufs=1) as c_pool,
            tc.tile_pool(name="acc", bufs=8, space="PSUM") as psum_pool,
        ):
            # Tiny constants first so they land with queue startup (the ones
            # tile is kept small — everything ahead of the x tiles on this
            # queue delays the cold-start x stream).
            ones_sb = c_pool.tile([128, 128], BF16, tag="ones")
            nc.scalar.dma_start(out=ones_sb[:], in_=ones[:])
            bias_sb = c_pool.tile([1, O_L], BF16, tag="bias")
            nc.scalar.dma_start(out=bias_sb[:], in_=bvec[:])

            # The first two W quads as separate [128, 512] tiles so early
            # matmuls only wait on 128 KB loads, and the first few x tiles
            # riding BOTH rings, interleaved on the SP ring in consumption
            # order (kt0 pairs with W chunk k4=0, kt1 with k4=1, ...) — the
            # cold-start window is paced entirely by these arrivals.
            def _w0_tile(ktq, k4):
                t = w0_pool.tile([128, 512], BF16, tag=f"w0_{ktq}_{k4}",
                                 name=f"w0_{ktq}_{k4}")
                nc.sync.dma_start(
                    out=t[:],
                    in_=w[ktq * 128 : (ktq + 1) * 128,
                          k4 * 512 : (k4 + 1) * 512],
                )
                return t

            xts = [None] * KT

            def _xt_tile(kt, eng):
                t = xt_pool.tile([128, B_L], BF16, tag=f"xt{kt}", name=f"xt{kt}")
                eng.dma_start(out=t[:], in_=xT[kt * 128 : (kt + 1) * 128, :])
                xts[kt] = t

            w0q = {}
            w0q[(0, 0)] = _w0_tile(0, 0)
            _xt_tile(0, nc.scalar)
            _xt_tile(1, nc.sync)
            w0q[(0, 1)] = _w0_tile(0, 1)
            _xt_tile(2, nc.scalar)
            _xt_tile(3, nc.sync)
            w0q[(0, 2)] = _w0_tile(0, 2)
            _xt_tile(4, nc.scalar)
            _xt_tile(5, nc.sync)
            w0q[(0, 3)] = _w0_tile(0, 3)
            for k4 in range(KQ):
                w0q[(1, k4)] = _w0_tile(1, k4)
            for kt in range(6, KT):
                _xt_tile(kt, nc.scalar)

            # Broadcast bias rows for the DVE drain; first needed when the
            # first output chunk finishes (~60 us), well after the x tiles
            # ahead of it on this ring.
            bbc_sb = c_pool.tile([128, O_L], F32, tag="bbc")
            nc.scalar.dma_start(out=bbc_sb[:], in_=bbc[:])

            # Warm-up matmuls on the tiny bias constant: the PE HAM clock
            # gate needs ~3.4 us of sustained activity to lift the cold
            # 1.2 GHz throttle, and the first real matmul can't start until
            # its x/W tiles land (~12 us: NEFF preamble + first loads).
            # Burning that idle window on dummy K=1 matmuls makes the real
            # stream run warm from its first instruction.
            # Full-K warmups: K=1 matmuls light only 1 of 128 PE rows and
            # barely register with the HAM activity monitor (flip came ~10us
            # after they started); full 128-row matmuls flip it on schedule.
            # 32 x N=128 covers the ~3.4us activity window.
            warm = psum_pool.tile([128, 512], F32, tag="acc", name="warm")
            for i in range(32):
                nc.tensor.matmul(
                    warm[:, 0:128],
                    ones_sb[:],
                    ones_sb[:],
                    start=(i == 0),
                    stop=(i == 31),
                )

            for oc in range(OC):
                accs = [
                    psum_pool.tile([128, 512], F32, tag="acc",
                                   name=f"acc_{oc}_{bt}")
                    for bt in range(BT)
                ]
                for ktq in range(NQ):
                    si = oc * NQ + ktq
                    if oc == 0 and ktq < 2:
                        wchunks = [w0q[(ktq, k4)][:] for k4 in range(KQ)]
                    else:
                        w_sl = w_pool.tile([128, KQ * 512], BF16, tag="wsl",
                                           name=f"w_{oc}_{ktq}")
                        nc.sync.dma_start(
                            out=w_sl[:], in_=w[si * 128 : (si + 1) * 128, :]
                        )
                        wchunks = [
                            w_sl[:, k4 * 512 : (k4 + 1) * 512] for k4 in range(KQ)
                        ]
                    if ktq < NQ - 1:
                        for k4 in range(KQ):
                            kt = ktq * KQ + k4
                            for bt in range(BT):
                                nc.tensor.matmul(
                                    accs[bt][:],
                                    xts[kt][:, bt * 128 : (bt + 1) * 128],
                                    wchunks[k4],
                                    start=(kt == 0),
                                    stop=False,
                                )
                    else:
                        # Last quad bt-major: each batch tile finishes its
                        # contraction and drains while the PE moves on —
                        # banks free one at a time, drains and stores
                        # overlap the next oc's matmuls.
                        #
                        # Mid-kernel, bias + relu happen on the (otherwise
                        # idle) DVE instead of costing PE matmuls, and the
                        # y stores stay OFF the sync queue: HWDGE queues
                        # are strictly in-order, so a store parked behind
                        # an unmet semaphore would block the next chunk's
                        # W-slab fetch behind it and stall the PE.
                        #
                        # For the final chunk there are no W fetches left
                        # to protect, but the drain chain IS the kernel
                        # tail — so spend 8 K=1 matmuls on the bias to keep
                        # the DVE drain single-op (0.7us < 0.86us matmul
                        # spacing) and split the stores across both queues.
                        last_oc = oc == OC - 1
                        y_sls = []
                        for bt in range(BT):
                            for k4 in range(KQ):
                                kt = ktq * KQ + k4
                                nc.tensor.matmul(
                                    accs[bt][:],
                                    xts[kt][:, bt * 128 : (bt + 1) * 128],
                                    wchunks[k4],
                                    start=False,
                                    stop=(not last_oc and kt == KT - 1),
                                )
                            y_sl = y_pool.tile([128, 512], BF16, tag="yst",
                                               name=f"y_{oc}_{bt}")
                            y_sls.append(y_sl)
                            if last_oc:
                                nc.tensor.matmul(
                                    accs[bt][:],
                                    ones_sb[0:1, 0:128],
                                    bias_sb[:, oc * 512 : (oc + 1) * 512],
                                    start=False,
                                    stop=True,
                                )
                                if bt < BT - 1:
                                    nc.vector.tensor_scalar_max(
                                        y_sl[:], accs[bt][:], 0.0
                                    )
                                    dma_eng = (
                                        nc.scalar if bt % 2 == 0 else nc.sync
                                    )
                                    dma_eng.dma_start(
                                        out=y[
                                            bt * 128 : (bt + 1) * 128,
                                            oc * 512 : (oc + 1) * 512,
                                        ],
                                        in_=y_sl[:],
                                    )
                                else:
                                    # Very last tile: halve the relu+store
                                    # and fan across both queues — this
                                    # chain IS the kernel tail.
                                    for h, dma_eng in enumerate(
                                        (nc.sync, nc.scalar)
                                    ):
                                        cols = slice(h * 256, (h + 1) * 256)
                                        nc.vector.tensor_scalar_max(
                                            y_sl[:, cols], accs[bt][:, cols], 0.0
                                        )
                                        dma_eng.dma_start(
                                            out=y[
                                                bt * 128 : (bt + 1) * 128,
                                                oc * 512 + h * 256 : oc * 512
                                                + (h + 1) * 256,
                                            ],
                                            in_=y_sl[:, cols],
                                        )
                            else:
                                # Bias-add on DVE frees the PSUM bank in
                                # 0.7us — under the 0.86us/bt matmul pace,
                                # so the next chunk's matmuls never wait on
                                # a bank. Only the adds sit on the DVE
                                # during the handoff; relu + store follow
                                # below once all banks are clear.
                                nc.vector.tensor_add(
                                    y_sl[:],
                                    accs[bt][:],
                                    bbc_sb[:, oc * 512 : (oc + 1) * 512],
                                )
                        if not last_oc:
                            for bt in range(BT):
                                nc.vector.tensor_scalar_max(
                                    y_sls[bt][:], y_sls[bt][:], 0.0
                                )
                                nc.scalar.dma_start(
                                    out=y[
                                        bt * 128 : (bt + 1) * 128,
                                        oc * 512 : (oc + 1) * 512,
                                    ],
                                    in_=y_sls[bt][:],
                                )

    nc.finalize()
    return nc


def _materialize_w(core0, core1, core2, core3) -> np.ndarray:
    """Contract the TT cores into the dense 4096x4096 matrix M.

    M[(m1 m2 m3 m4), (n1 n2 n3 n4)] (big-endian mode order on both sides),
    matching the reference's x/y index conventions.
    """
    g1 = core0[0].astype(np.float64)            # [m1, n1, r1]
    t12 = np.einsum("mnr,rMNs->mMnNs", g1, core1.astype(np.float64))
    a12 = t12.reshape(64, 64, 8)                # [(m1 m2), (n1 n2), r2]
    g4 = core3[..., 0].astype(np.float64)       # [r3, m4, n4]
    t34 = np.einsum("rmns,sMN->rmMnN", core2.astype(np.float64), g4)
    b34 = t34.reshape(8, 64, 64)                # [r2, (m3 m4), (n3 n4)]
    w = np.einsum("mnr,rMN->mMnN", a12, b34)    # [(m12),(m34),(n12),(n34)]
    return np.ascontiguousarray(
        w.reshape(F_FULL, O_FULL), dtype=np.float32
    )


def _prepare_in_maps(x, w_full, bias):
    """Shard + bf16-quantize host-side. Core c = (g, h): batch group g,
    output-column group h."""
    xts = [
        np.ascontiguousarray(
            x[g * B_L : (g + 1) * B_L, :].T.astype(NP_BF16)
        )
        for g in range(BG)
    ]
    ws = []
    for h in range(OG):
        wh = w_full[:, h * O_L : (h + 1) * O_L].astype(NP_BF16)
        t = wh.reshape(NQ, KQ, 128, OC, 512)      # [ktq, k4, p, oc, c]
        wb = np.ascontiguousarray(t.transpose(3, 0, 2, 1, 4)).reshape(
            F_FULL, O_L
        )                                          # [(oc ktq p), (k4 c)]
        ws.append(wb)
    bs = [
        np.ascontiguousarray(bias[:, h * O_L : (h + 1) * O_L].astype(NP_BF16))
        for h in range(OG)
    ]
    bbcs = [
        np.ascontiguousarray(
            np.broadcast_to(bias[:, h * O_L : (h + 1) * O_L], (128, O_L))
        ).astype(np.float32)
        for h in range(OG)
    ]
    ones = np.ones((128, 640), dtype=NP_BF16)
    in_maps = []
    for c in range(N_CORES):
        g, h = divmod(c, OG)
        in_maps.append(
            {"xT": xts[g], "w": ws[h], "bvec": bs[h], "ones": ones, "bbc": bbcs[h]}
        )
    return in_maps


def kernel(x, core0, core1, core2, core3, b) -> np.ndarray:
    x = np.asarray(x, dtype=np.float32)
    w_full = _materialize_w(
        np.asarray(core0, dtype=np.float32),
        np.asarray(core1, dtype=np.float32),
        np.asarray(core2, dtype=np.float32),
        np.asarray(core3, dtype=np.float32),
    )
    bias = np.asarray(b, dtype=np.float32).reshape(1, O_FULL)

    if "nc" not in _CACHE:
        _CACHE["nc"] = _build_module()
    nc = _CACHE["nc"]

    in_maps = _prepare_in_maps(x, w_full, bias)
    res = run_bass_kernel_spmd(nc, in_maps, core_ids=list(range(N_CORES)))

    y = np.empty((B_FULL, O_FULL), dtype=np.float32)
    for c in range(N_CORES):
        g, h = divmod(c, OG)
        y[g * B_L : (g + 1) * B_L, h * O_L : (h + 1) * O_L] = np.asarray(
            res.results[c]["y"]
        ).astype(np.float32)
    return y


# revision 34
# speedup vs baseline: 1.2362x; 1.0256x over previous
"""Trainium2 kernel for nn_KerasDense_32263794328408.

y = relu(x @ M + b), where M is a 4096x4096 TT-matrix (cores of shape
[r_{k-1}, 8, 8, r_k], ranks [1,8,8,8,1]).

Strategy: the TT cores are tiny (<17 KB each); materialize the dense
M = 4096x4096 on the host (cheap, ~270 MFLOP) and run the dense
y = relu(x @ M + b) as a near-roofline GEMM on 8 NeuronCores.

Sharding: 2D grid, 4 batch groups x 2 output-column groups.
Per core: x-shard [1024, 4096] (shipped transposed as xT [4096, 1024]),
W column-half [4096, 2048] and bias half, producing y [1024, 2048].

Inputs are shipped in bf16 (quantization absmax/scale ~2.5e-3, well
under the 2e-2 gate). vs fp32r this halves HBM traffic — the f32
version was pinned at the per-core HBM ceiling during the x-load phase
and stalled the PE — and bf16 LDWEIGHTS gets fast-weight-load, so the
stationary reload hides fully under the matmul stream.

On-chip: x-stationary matmuls. lhsT = xT tile [128k x 128b], rhs = W
slab [128k x 512o], PSUM accumulation over the 32 k-tiles plus a K=1
matmul adding the bias. W is host-pre-blocked so every slab DMA is a
fully contiguous [128, 2048] read (4 KB per partition line). The last
k-quad of each output chunk runs bt-major with the bias matmul and the
DVE relu+store chained per batch tile, so PSUM banks free one by one
and the drains overlap the next chunk's matmuls (short kernel tail).
"""

import sys

if "/opt/trn_rl_repo" not in sys.path:
    sys.path.insert(0, "/opt/trn_rl_repo")

import ml_dtypes
import numpy as np

import concourse.bacc as bacc
import concourse.bass as bass
import concourse.mybir as mybir
import concourse.tile as tile
from concourse.bass_utils import run_bass_kernel_spmd

F32 = mybir.dt.float32
BF16 = mybir.dt.bfloat16
F8E4 = mybir.dt.float8e4
NP_BF16 = ml_dtypes.bfloat16
NP_F8 = mybir.dt.np(F8E4)

# One k-tile pair computed as a single fp8-e4m3 DoubleRow matmul (contracts
# K=256 in one pass at ~2x rate). Quantizing 2 of 32 k-tiles to fp8 raises
# absmax/scale from ~3.9e-3 to ~1.1e-2 (gate 2e-2) and saves ~6us of PE time.
KP = 24  # pair covers kt 24 and 25 (ktq 6, k4 0..1)

B_FULL = 4096  # batch
F_FULL = 4096  # input features
O_FULL = 4096  # output features

BG = 4  # batch groups
OG = 2  # output-column groups
N_CORES = BG * OG

B_L = B_FULL // BG   # 1024 batch rows per core
O_L = O_FULL // OG   # 2048 output cols per core
KT = F_FULL // 128   # 32 contraction tiles
OC = O_L // 512      # 4 output chunks of 512 per core
BT = B_L // 128      # 8 batch tiles of 128 per core
KQ = 4               # k-tiles fetched per W DMA
NQ = KT // KQ        # 8 quad fetches per oc

_CACHE: dict = {}


def _build_module() -> bass.Bass:
    nc = bacc.Bacc(None, target_bir_lowering=False)

    xT = nc.declare_dram_parameter("xT", [F_FULL, B_L], BF16, isOutput=False)
    # w is host-pre-blocked: row si*128+p, col k4*512+c holds
    # W[(ktq*4+k4)*128 + p, oc*512 + c] with si = oc*NQ + ktq, so each
    # slab (oc, ktq) is one contiguous [128, 2048] read.
    w = nc.declare_dram_parameter("w", [F_FULL, O_L], BF16, isOutput=False)
    bvec = nc.declare_dram_parameter("bvec", [1, O_L], BF16, isOutput=False)
    ones = nc.declare_dram_parameter("ones", [128, 128], BF16, isOutput=False)
    bbc = nc.declare_dram_parameter("bbc", [128, O_L], F32, isOutput=False)
    # fp8 DoubleRow pair operands: x8[p, i*B_L + b] = e4m3(xT[(KP+i)*128+p, b]),
    # w8[oc*128 + p, i*512 + c] = e4m3(W[(KP+i)*128+p, oc*512+c]).
    x8 = nc.declare_dram_parameter("x8", [128, 2 * B_L], F8E4, isOutput=False)
    w8 = nc.declare_dram_parameter("w8", [OC * 128, 1024], F8E4, isOutput=False)
    # y ships back as bf16 (host upcasts): halves store traffic and the
    # tail store; costs ~2e-3 extra absmax, still 5x under the gate.
    y = nc.declare_dram_parameter("y", [B_L, O_L], BF16, isOutput=True)

    with tile.TileContext(nc) as tc:
        with (
            tc.tile_pool(name="xt", bufs=1) as xt_pool,
            tc.tile_pool(name="w0", bufs=1) as w0_pool,
            tc.tile_pool(name="wsl", bufs=6) as w_pool,
            tc.tile_pool(name="yst", bufs=12) as y_pool,
            tc.tile_pool(name="cst", bufs=1) as c_pool,
            tc.tile_pool(name="acc", bufs=8, space="PSUM") as psum_pool,
        ):
            # Tiny constants first so they land with queue startup (the ones
            # tile is kept small — everything ahead of the x tiles on this
            # queue delays the cold-start x stream).
            ones_sb = c_pool.tile([128, 128], BF16, tag="ones")
            nc.scalar.dma_start(out=ones_sb[:], in_=ones[:])
            bias_sb = c_pool.tile([1, O_L], BF16, tag="bias")
            nc.scalar.dma_start(out=bias_sb[:], in_=bvec[:])

            # The first two W quads as separate [128, 512] tiles so early
            # matmuls only wait on 128 KB loads, and the first few x tiles
            # riding BOTH rings, interleaved on the SP ring in consumption
            # order (kt0 pairs with W chunk k4=0, kt1 with k4=1, ...) — the
            # cold-start window is paced entirely by these arrivals.
            def _w0_tile(ktq, k4):
                t = w0_pool.tile([128, 512], BF16, tag=f"w0_{ktq}_{k4}",
                                 name=f"w0_{ktq}_{k4}")
                nc.sync.dma_start(
                    out=t[:],
                    in_=w[ktq * 128 : (ktq + 1) * 128,
                          k4 * 512 : (k4 + 1) * 512],
                )
                return t

            xts = [None] * KT

            def _xt_tile(kt, eng):
                t = xt_pool.tile([128, B_L], BF16, tag=f"xt{kt}", name=f"xt{kt}")
                eng.dma_start(out=t[:], in_=xT[kt * 128 : (kt + 1) * 128, :])
                xts[kt] = t

            w0q = {}
            w0q[(0, 0)] = _w0_tile(0, 0)
            _xt_tile(0, nc.scalar)
            _xt_tile(1, nc.sync)
            w0q[(0, 1)] = _w0_tile(0, 1)
            _xt_tile(2, nc.scalar)
            _xt_tile(3, nc.sync)
            w0q[(0, 2)] = _w0_tile(0, 2)
            _xt_tile(4, nc.scalar)
            _xt_tile(5, nc.sync)
            w0q[(0, 3)] = _w0_tile(0, 3)
            for k4 in range(KQ):
                w0q[(1, k4)] = _w0_tile(1, k4)
            for kt in range(6, KT):
                _xt_tile(kt, nc.scalar)

            # fp8 pair stationary (first needed at ~55us, after the x tiles).
            x8_sb = c_pool.tile([128, 2 * B_L], F8E4, tag="x8")
            nc.scalar.dma_start(out=x8_sb[:], in_=x8[:])

            # Broadcast bias rows for the DVE drain; first needed when the
            # first output chunk finishes (~60 us), well after the x tiles
            # ahead of it on this ring.
            bbc_sb = c_pool.tile([128, O_L], F32, tag="bbc")
            nc.scalar.dma_start(out=bbc_sb[:], in_=bbc[:])

            # Warm-up matmuls on the tiny bias constant: the PE HAM clock
            # gate needs ~3.4 us of sustained activity to lift the cold
            # 1.2 GHz throttle, and the first real matmul can't start until
            # its x/W tiles land (~12 us: NEFF preamble + first loads).
            # Burning that idle window on dummy K=1 matmuls makes the real
            # stream run warm from its first instruction.
            # Full-K warmups: K=1 matmuls light only 1 of 128 PE rows and
            # barely register with the HAM activity monitor (flip came ~10us
            # after they started); full 128-row matmuls flip it on schedule.
            # 32 x N=128 covers the ~3.4us activity window.
            warm = psum_pool.tile([128, 512], F32, tag="acc", name="warm")
            for i in range(32):
                nc.tensor.matmul(
                    warm[:, 0:128],
                    ones_sb[:],
                    ones_sb[:],
                    start=(i == 0),
                    stop=(i == 31),
                )

            for oc in range(OC):
                accs = [
                    psum_pool.tile([128, 512], F32, tag="acc",
                                   name=f"acc_{oc}_{bt}")
                    for bt in range(BT)
                ]
                for ktq in range(NQ):
                    si = oc * NQ + ktq
                    if oc == 0 and ktq < 2:
                        wchunks = [w0q[(ktq, k4)][:] for k4 in range(KQ)]
                    else:
                        w_sl = w_pool.tile([128, KQ * 512], BF16, tag="wsl",
                                           name=f"w_{oc}_{ktq}")
                        nc.sync.dma_start(
                            out=w_sl[:], in_=w[si * 128 : (si + 1) * 128, :]
                        )
                        wchunks = [
                            w_sl[:, k4 * 512 : (k4 + 1) * 512] for k4 in range(KQ)
                        ]
                    if ktq == KP // KQ:
                        # kt 24+25 as one fp8 DoubleRow matmul per bt
                        # (3D APs [p, 2, free]; out = sum of both k-planes),
                        # then kt 26,27 in bf16 from the regular slab.
                        w8_sb = w_pool.tile([128, 1024], F8E4, tag="w8",
                                            name=f"w8_{oc}")
                        nc.sync.dma_start(
                            out=w8_sb[:],
                            in_=w8[oc * 128 : (oc + 1) * 128, :],
                        )
                        x8_3d = x8_sb[:].rearrange("p (i b) -> p i b", i=2)
                        w8_3d = w8_sb[:].rearrange("p (i c) -> p i c", i=2)
                        for bt in range(BT):
                            nc.tensor.matmul(
                                accs[bt][:],
                                x8_3d[:, :, bt * 128 : (bt + 1) * 128],
                                w8_3d,
                                start=False,
                                stop=False,
                                perf_mode=mybir.MatmulPerfMode.DoubleRow,
                            )
                        for k4 in (2, 3):
                            kt = ktq * KQ + k4
                            for bt in range(BT):
                                nc.tensor.matmul(
                                    accs[bt][:],
                                    xts[kt][:, bt * 128 : (bt + 1) * 128],
                                    wchunks[k4],
                                    start=False,
                                    stop=False,
                                )
                    elif ktq < NQ - 1:
                        for k4 in range(KQ):
                            kt = ktq * KQ + k4
                            for bt in range(BT):
                                nc.tensor.matmul(
                                    accs[bt][:],
                                    xts[kt][:, bt * 128 : (bt + 1) * 128],
                                    wchunks[k4],
                                    start=(kt == 0),
                                    stop=False,
                                )
                    else:
                        # Last quad bt-major: each batch tile finishes its
                        # contraction and drains while the PE moves on —
                        # banks free one at a time, drains and stores
                        # overlap the next oc's matmuls.
                        #
                        # Mid-kernel, bias + relu happen on the (otherwise
                        # idle) DVE instead of costing PE matmuls, and the
                        # y stores stay OFF the sync queue: HWDGE queues
                        # are strictly in-order, so a store parked behind
                        # an unmet semaphore would block the next chunk's
                        # W-slab fetch behind it and stall the PE.
                        #
                        # For the final chunk there are no W fetches left
                        # to protect, but the drain chain IS the kernel
                        # tail — so spend 8 K=1 matmuls on the bias to keep
                        # the DVE drain single-op (0.7us < 0.86us matmul
                        # spacing) and split the stores across both queues.
                        last_oc = oc == OC - 1
                        y_sls = []
                        for bt in range(BT):
                            for k4 in range(KQ):
                                kt = ktq * KQ + k4
                                nc.tensor.matmul(
                                    accs[bt][:],
                                    xts[kt][:, bt * 128 : (bt + 1) * 128],
                                    wchunks[k4],
                                    start=False,
                                    stop=(not last_oc and kt == KT - 1),
                                )
                            y_sl = y_pool.tile([128, 512], BF16, tag="yst",
                                               name=f"y_{oc}_{bt}")
                            y_sls.append(y_sl)
                            if last_oc:
                                nc.tensor.matmul(
                                    accs[bt][:],
                                    ones_sb[0:1, 0:128],
                                    bias_sb[:, oc * 512 : (oc + 1) * 512],
                                    start=False,
                                    stop=True,
                                )
                                if bt < BT - 1:
                                    nc.vector.tensor_scalar_max(
                                        y_sl[:], accs[bt][:], 0.0
                                    )
                                    dma_eng = (
                                        nc.scalar if bt % 2 == 0 else nc.sync
                                    )
                                    dma_eng.dma_start(
                                        out=y[
                                            bt * 128 : (bt + 1) * 128,
                                            oc * 512 : (oc + 1) * 512,
                                        ],
                                        in_=y_sl[:],
                                    )
                                else:
                                    # Very last tile: halve the relu+store
                                    # and fan across both queues — this
                                    # chain IS the kernel tail.
                                    for h, dma_eng in enumerate(
                                        (nc.sync, nc.scalar)
                                    ):
                                        cols = slice(h * 256, (h + 1) * 256)
                                        nc.vector.tensor_scalar_max(
                                            y_sl[:, cols], accs[bt][:, cols], 0.0
                                        )
                                        dma_eng.dma_start(
                                            out=y[
                                                bt * 128 : (bt + 1) * 128,
                                                oc * 512 + h * 256 : oc * 512
                                                + (h + 1) * 256,
                                            ],
                                            in_=y_sl[:, cols],
                                        )
                            else:
                                # Bias-add on DVE frees the PSUM bank in
                                # 0.7us — under the 0.86us/bt matmul pace,
                                # so the next chunk's matmuls never wait on
                                # a bank. Only the adds sit on the DVE
                                # during the handoff; relu + store follow
                                # below once all banks are clear.
                                nc.vector.tensor_add(
                                    y_sl[:],
                                    accs[bt][:],
                                    bbc_sb[:, oc * 512 : (oc + 1) * 512],
                                )
                        if not last_oc:
                            for bt in range(BT):
                                nc.vector.tensor_scalar_max(
                                    y_sls[bt][:], y_sls[bt][:], 0.0
                                )
                                nc.scalar.dma_start(
                                    out=y[
                                        bt * 128 : (bt + 1) * 128,
                                        oc * 512 : (oc + 1) * 512,
                                    ],
                                    in_=y_sls[bt][:],
                                )

    nc.finalize()
    return nc


def _materialize_w(core0, core1, core2, core3) -> np.ndarray:
    """Contract the TT cores into the dense 4096x4096 matrix M.

    M[(m1 m2 m3 m4), (n1 n2 n3 n4)] (big-endian mode order on both sides),
    matching the reference's x/y index conventions.
    """
    g1 = core0[0].astype(np.float64)            # [m1, n1, r1]
    t12 = np.einsum("mnr,rMNs->mMnNs", g1, core1.astype(np.float64))
    a12 = t12.reshape(64, 64, 8)                # [(m1 m2), (n1 n2), r2]
    g4 = core3[..., 0].astype(np.float64)       # [r3, m4, n4]
    t34 = np.einsum("rmns,sMN->rmMnN", core2.astype(np.float64), g4)
    b34 = t34.reshape(8, 64, 64)                # [r2, (m3 m4), (n3 n4)]
    w = np.einsum("mnr,rMN->mMnN", a12, b34)    # [(m12),(m34),(n12),(n34)]
    return np.ascontiguousarray(
        w.reshape(F_FULL, O_FULL), dtype=np.float32
    )


def _prepare_in_maps(x, w_full, bias):
    """Shard + bf16-quantize host-side. Core c = (g, h): batch group g,
    output-column group h."""
    xts = [
        np.ascontiguousarray(
            x[g * B_L : (g + 1) * B_L, :].T.astype(NP_BF16)
        )
        for g in range(BG)
    ]
    ws = []
    for h in range(OG):
        wh = w_full[:, h * O_L : (h + 1) * O_L].astype(NP_BF16)
        t = wh.reshape(NQ, KQ, 128, OC, 512)      # [ktq, k4, p, oc, c]
        wb = np.ascontiguousarray(t.transpose(3, 0, 2, 1, 4)).reshape(
            F_FULL, O_L
        )                                          # [(oc ktq p), (k4 c)]
        ws.append(wb)
    bs = [
        np.ascontiguousarray(bias[:, h * O_L : (h + 1) * O_L].astype(NP_BF16))
        for h in range(OG)
    ]
    bbcs = [
        np.ascontiguousarray(
            np.broadcast_to(bias[:, h * O_L : (h + 1) * O_L], (128, O_L))
        ).astype(np.float32)
        for h in range(OG)
    ]
    ones = np.ones((128, 128), dtype=NP_BF16)
    # fp8 DoubleRow pair operands for kt KP, KP+1.
    x8s = []
    for g in range(BG):
        xTg = x[g * B_L : (g + 1) * B_L, :].T  # [F_FULL, B_L] f32
        pair = np.stack(
            [xTg[(KP + i) * 128 : (KP + i + 1) * 128, :] for i in range(2)],
            axis=1,
        )  # [128, 2, B_L]
        x8s.append(np.ascontiguousarray(pair.reshape(128, 2 * B_L)).astype(NP_F8))
    w8s = []
    for h in range(OG):
        wh = w_full[:, h * O_L : (h + 1) * O_L]  # [F_FULL, O_L] f32
        pair = np.stack(
            [wh[(KP + i) * 128 : (KP + i + 1) * 128, :] for i in range(2)],
            axis=1,
        )  # [128, 2, O_L]
        blocks = [
            pair[:, :, oc * 512 : (oc + 1) * 512].reshape(128, 1024)
            for oc in range(OC)
        ]
        w8s.append(
            np.ascontiguousarray(np.concatenate(blocks, axis=0)).astype(NP_F8)
        )
    in_maps = []
    for c in range(N_CORES):
        g, h = divmod(c, OG)
        in_maps.append(
            {
                "xT": xts[g],
                "w": ws[h],
                "bvec": bs[h],
                "ones": ones,
                "bbc": bbcs[h],
                "x8": x8s[g],
                "w8": w8s[h],
            }
        )
    return in_maps


def kernel(x, core0, core1, core2, core3, b) -> np.ndarray:
    x = np.asarray(x, dtype=np.float32)
    w_full = _materialize_w(
        np.asarray(core0, dtype=np.float32),
        np.asarray(core1, dtype=np.float32),
        np.asarray(core2, dtype=np.float32),
        np.asarray(core3, dtype=np.float32),
    )
    bias = np.asarray(b, dtype=np.float32).reshape(1, O_FULL)

    if "nc" not in _CACHE:
        _CACHE["nc"] = _build_module()
    nc = _CACHE["nc"]

    in_maps = _prepare_in_maps(x, w_full, bias)
    res = run_bass_kernel_spmd(nc, in_maps, core_ids=list(range(N_CORES)))

    y = np.empty((B_FULL, O_FULL), dtype=np.float32)
    for c in range(N_CORES):
        g, h = divmod(c, OG)
        y[g * B_L : (g + 1) * B_L, h * O_L : (h + 1) * O_L] = np.asarray(
            res.results[c]["y"]
        ).astype(np.float32)
    return y
